# revision 18
# baseline (speedup 1.0000x reference)
"""GAT 3-layer (DiffusionOrderingNetwork) Trainium2 kernel, 8-core SPMD.

Strategy: nodes partitioned 8x2500 by dst; per-core ELL (degree-sorted,
per-tile width) edge layout; per-edge gathers via dma_gather 256B tokens
from a shared HBM node table [20480, 64] whose row layout is
[payload(36) | 1.0 | al_src(6) | al_dst(6) | pad]; each core builds only
its own 2560-row block and an AllGather assembles the full table per
layer. Self-loops are NOT materialized as edge slots: each dst's
self-contribution is computed locally from the core's own table rows.
Segment softmax uses a constant-shift exp (exact after normalization);
padding slots point at a NEG pad row whose al_src is -1e4 in the HBM
copy only, so their exp is exactly 0. The constant 1.0 table column
makes the layer-3 slot tree-reduction produce the softmax denominator
for free. Per-tile aggregation is an in-place contiguous halving tree
on DVE (no strided tensor_reduce). The next layer's table block is
built inline per tile (PE transpose + matmul on idle engines).

Host path: one jitted SPMD dispatch per run (donated output buffer is
recycled run-to-run; no separate zero-buffer dispatch), inputs are three
packed buffers per core (i16 idx stream, bf16 x+W1, bf16 weights), the
bf16 output shards are fetched concurrently. Reported HW exec time is
the NTFF (neuron-profile) max-core device execution time; the
steady-state host end-to-end time is also measured and reported.
"""

import sys

sys.path.insert(0, "/opt/trn_rl_repo")

import numpy as np
import concourse.bass as bass
import concourse.bacc as bacc
import concourse.mybir as mybir
import concourse.tile as tile
from concourse import library_config

N = 20000
NC = 8
NPC = 2500          # nodes per core
NT = 20             # node tiles per core (128 rows each)
NPP = NT * 128      # 2560 padded nodes per core
NTBL = NC * NPP     # 20480 table rows
H = 6
D_IN = 32
HC = 36             # heads * hidden
TW = 64             # table row width (f32) = 256B gather token
RW = 49             # used row width: 36 pay | 1.0 | 6 al_src | 6 al_dst
F32 = mybir.dt.float32
I16 = mybir.dt.int16
BF16 = mybir.dt.bfloat16

CBF = NPP + 48      # bf16 blob: xT_own | Wc1
BBF_XT = 0
BBF_WC1 = NPP
CW = 48 + 48 + 192 + 104  # bf16 weights blob: Wc2 | Wc3 | W3 | bias row
W_WC2 = 0
W_WC3 = 48
W_W3 = 96
W_B = 288

CW3 = 32            # layer-3 slot chunk for the outer-product buffer

NEGP = 96           # NEG pad rows live at partitions 96:128 of last tile
PADT = NT - 1


def _blockdiag(a):
    Hh, C = a.shape
    out = np.zeros((Hh * C, Hh), np.float32)
    for h in range(Hh):
        out[h * C:(h + 1) * C, h] = a[h]
    return out


def _prep(x, edge_index, W1, a_src1, a_dst1, W2, a_src2, a_dst2, W3, a_src3,
          a_dst3, b1, b2, b3):
    src = np.asarray(edge_index[0], np.int64)
    dst = np.asarray(edge_index[1], np.int64)

    deg = np.bincount(dst, minlength=N)
    orders = []          # per core: local pos -> global node id
    global_pos = np.zeros(N, np.int64)
    for k in range(NC):
        d = deg[k * NPC:(k + 1) * NPC]
        order = np.argsort(-d, kind="stable") + k * NPC
        orders.append(order)
        global_pos[order] = k * NPP + np.arange(NPC)

    # shared tile width schedule (max across cores), exact widths
    Wt = np.zeros(NT, np.int64)
    for k in range(NC):
        ds_ = np.sort(deg[k * NPC:(k + 1) * NPC])[::-1]
        ds_ = np.concatenate([ds_, np.zeros(NPP - NPC, np.int64)])
        Wt = np.maximum(Wt, ds_.reshape(NT, 128).max(axis=1))
    Wt = np.maximum(Wt, 1)

    # CSR by dst
    sort_by_dst = np.argsort(dst, kind="stable")
    src_s = src[sort_by_dst]
    rowptr = np.zeros(N + 1, np.int64)
    np.cumsum(deg, out=rowptr[1:])

    # ELL index stream per core: [16, CIDX] i16, pads -> NEG row
    idx_w = []
    for k in range(NC):
        order = orders[k]
        neg = k * NPP + PADT * 128 + NEGP
        iw_parts = []
        for t in range(NT):
            w = int(Wt[t])
            ell = np.full((128, w), neg, np.int64)
            for p in range(128):
                li = t * 128 + p
                if li < NPC:
                    n = order[li]
                    e0, e1 = rowptr[n], rowptr[n + 1]
                    ell[p, :e1 - e0] = global_pos[src_s[e0:e1]]
            stream = ell.T.reshape(-1)            # slot-major: s*128+p
            iw_parts.append(stream.reshape(-1, 16).T)  # [16, 8w]
        idx_w.append(np.concatenate(iw_parts, axis=1).astype(np.int16))

    # x in permuted order, padded, transposed: per-core [32, NPP]
    import ml_dtypes
    Wc1 = np.concatenate([W1, W1 @ _blockdiag(a_src1), W1 @ _blockdiag(a_dst1)], 1)
    Wc2 = np.concatenate([W2, W2 @ _blockdiag(a_src2), W2 @ _blockdiag(a_dst2)], 1)
    I36 = np.eye(HC, dtype=np.float32)
    Wc3 = np.concatenate([I36, W3 @ _blockdiag(a_src3), W3 @ _blockdiag(a_dst3)], 1)

    blobbf = []
    for k in range(NC):
        xp = np.zeros((NPP, D_IN), np.float32)
        xp[:NPC] = x[orders[k]]
        b = np.zeros((D_IN, CBF), ml_dtypes.bfloat16)
        b[:, BBF_XT:BBF_XT + NPP] = np.ascontiguousarray(xp.T).astype(ml_dtypes.bfloat16)
        b[:, BBF_WC1:BBF_WC1 + 48] = Wc1.astype(ml_dtypes.bfloat16)
        blobbf.append(b)

    wblob = np.zeros((HC, CW), ml_dtypes.bfloat16)
    wblob[:, W_WC2:W_WC2 + 48] = Wc2.astype(ml_dtypes.bfloat16)
    wblob[:, W_WC3:W_WC3 + 48] = Wc3.astype(ml_dtypes.bfloat16)
    wblob[:, W_W3:W_W3 + 192] = W3.astype(ml_dtypes.bfloat16)
    wblob[0, W_B:W_B + HC] = b1.astype(ml_dtypes.bfloat16)
    wblob[0, W_B + HC:W_B + 2 * HC] = b2.astype(ml_dtypes.bfloat16)
    wblob[0, W_B + 2 * HC:W_B + 2 * HC + D_IN] = b3.astype(ml_dtypes.bfloat16)

    unperm = np.concatenate(orders)  # row i of stacked core outputs -> node id
    return Wt, idx_w, blobbf, wblob, unperm


def _tree_add(nc, buf3, w):
    """In-place halving-tree sum over the middle (slot) axis of a
    [128, w, S] view; result lands in slot 0. Contiguous DVE adds."""
    while w > 1:
        k = w // 2
        nc.vector.tensor_tensor(out=buf3[:, 0:k, :], in0=buf3[:, 0:k, :],
                                in1=buf3[:, w - k:w, :],
                                op=mybir.AluOpType.add)
        w -= k


def _build(nc, Wt):
    SWt = int(Wt.sum())
    CIDX = 8 * SWt

    t_idx = nc.dram_tensor("idxw", [16, CIDX], I16, kind="ExternalInput")
    t_bf = nc.dram_tensor("bbf", [D_IN, CBF], BF16, kind="ExternalInput")
    t_w = nc.dram_tensor("wbl", [HC, CW], BF16, kind="ExternalInput")
    t_out = nc.dram_tensor("out", [NPP, D_IN], BF16, kind="ExternalOutput")

    with tile.TileContext(nc) as tc:
        with (
            tc.tile_pool(name="dram", bufs=1, space="DRAM") as dram,
            tc.tile_pool(name="cst", bufs=1) as cst,
            tc.tile_pool(name="gat", bufs=4) as gat,
            tc.tile_pool(name="wrk", bufs=3) as wrk,
            tc.tile_pool(name="big", bufs=2) as big,
            tc.tile_pool(name="acc", bufs=1) as acc,
            tc.tile_pool(name="ps", bufs=2, space="PSUM") as ps,
        ):
            nc.gpsimd.load_library(library_config.mlp)
            TBLS = [dram.tile([NTBL, TW], F32, addr_space="Shared",
                              name=f"tbls{i}", tag=f"tbls{i}") for i in range(3)]
            BNC = dram.tile([NPP, TW], F32)
            CCI = dram.tile([32, 1], F32)
            CCO = dram.tile([32, 1], F32)

            # idx stream: load once, replicate to 128 partitions on-chip
            sb_idx = cst.tile([128, CIDX], I16)
            nc.sync.dma_start(out=sb_idx[0:16, :], in_=t_idx[:])
            nc.sync.dma_start(out=sb_idx[16:32, :], in_=sb_idx[0:16, :])
            nc.sync.dma_start(out=sb_idx[32:64, :], in_=sb_idx[0:32, :])
            nc.sync.dma_start(out=sb_idx[64:128, :], in_=sb_idx[0:64, :])

            sb_bf = cst.tile([D_IN, CBF], BF16)
            nc.sync.dma_start(out=sb_bf[:], in_=t_bf[:])
            sb_w = cst.tile([HC, CW], BF16)
            nc.sync.dma_start(out=sb_w[:], in_=t_w[:])

            sb_xTo = sb_bf[:, BBF_XT:BBF_XT + NPP]
            sb_wc = [sb_bf[:, BBF_WC1:BBF_WC1 + 48],
                     sb_w[:, W_WC2:W_WC2 + 48],
                     sb_w[:, W_WC3:W_WC3 + 48]]
            sb_w3 = sb_w[:, W_W3:W_W3 + 192]

            # on-device constants: identity, row mask, biases
            eye = cst.tile([128, 128], F32)
            nc.gpsimd.memset(eye[:], 0.0)
            nc.gpsimd.affine_select(out=eye[:], in_=eye[:],
                                    compare_op=mybir.AluOpType.not_equal,
                                    fill=1.0, base=0, pattern=[[-1, 128]],
                                    channel_multiplier=1)
            rm = cst.tile([128, NT], F32)
            nc.gpsimd.memset(rm[:], 1.0)
            # keep where (NPC-1) - p - 128*t >= 0, i.e. real rows
            nc.gpsimd.affine_select(out=rm[:], in_=rm[:],
                                    compare_op=mybir.AluOpType.is_ge,
                                    fill=0.0, base=NPC - 1,
                                    pattern=[[-128, NT]],
                                    channel_multiplier=-1)
            ones1b = cst.tile([1, 128], BF16)
            nc.vector.memset(ones1b[:], 1.0)
            bias_ps = ps.tile([128, 104], F32, tag="tb")
            nc.tensor.matmul(bias_ps[:], ones1b[:], sb_w[0:1, W_B:W_B + 104],
                             start=True, stop=True)
            sb_bias = cst.tile([128, 104], F32)
            nc.scalar.activation(sb_bias[:], bias_ps[:],
                                 mybir.ActivationFunctionType.Copy)
            sb_b = [sb_bias[:, 0:HC], sb_bias[:, HC:2 * HC],
                    sb_bias[:, 2 * HC:2 * HC + D_IN]]

            bm20 = cst.tile([128, 1], F32)
            nc.vector.memset(bm20[:], -20.0)
            bm50 = cst.tile([128, 1], F32)
            nc.vector.memset(bm50[:], -50.0)

            # persistent own-rows table blocks (ping-pong across layers);
            # col 36 is the constant 1.0 column (written once, never touched)
            TBO = [acc.tile([128, NT, RW], F32, tag="tbo0", name="tbo0"),
                   acc.tile([128, NT, RW], F32, tag="tbo1", name="tbo1")]
            nc.vector.memset(TBO[0][:, :, 36], 1.0)
            nc.vector.memset(TBO[1][:, :, 36], 1.0)
            e3_all = acc.tile([128, NT, D_IN], F32)

            def store_own(dstbuf, t, pt):
                # psum row [pay(36) | al_src(6) | al_dst(6)] -> RW layout
                nc.scalar.activation(dstbuf[:, t, 0:36], pt[:, 0:36],
                                     mybir.ActivationFunctionType.Copy)
                nc.scalar.activation(dstbuf[:, t, 37:49], pt[:, 36:48],
                                     mybir.ActivationFunctionType.Copy)

            def emit_tile(dstbuf, t):
                # own block row -> BNC (HBM); NEG pad rows get al_src=-1e4
                # in the HBM copy only (local copy stays finite)
                if t == PADT:
                    sb_tb = wrk.tile([128, RW], F32, tag="tbpatch")
                    nc.vector.tensor_copy(sb_tb[:], dstbuf[:, t, :])
                    nc.vector.memset(sb_tb[NEGP:128, 37:43], -10000.0)
                    nc.sync.dma_start(out=BNC[t * 128:(t + 1) * 128, 0:RW],
                                      in_=sb_tb[:])
                else:
                    nc.sync.dma_start(out=BNC[t * 128:(t + 1) * 128, 0:RW],
                                      in_=dstbuf[:, t, :])

            # layer-1 table: xT (bf16) @ Wc1
            for t in range(NT):
                pt = ps.tile([128, 48], F32, tag="tb")
                nc.tensor.matmul(pt[:], sb_xTo[:, t * 128:(t + 1) * 128],
                                 sb_wc[0][:], start=True, stop=True)
                store_own(TBO[0], t, pt)
                emit_tile(TBO[0], t)
            tc.strict_bb_all_engine_barrier()
            nc.gpsimd.collective_compute(
                "AllGather", mybir.AluOpType.bypass,
                replica_groups=[list(range(NC))],
                ins=[BNC[:].opt()], outs=[TBLS[0][:].opt()])
            tc.strict_bb_all_engine_barrier()

            qctr = 0
            for li in range(3):
                cur = TBO[li % 2]
                nxt = TBO[(li + 1) % 2]
                ioff = 0
                for t in range(NT):
                    w = int(Wt[t])
                    G = gat.tile([128, w, TW], F32, tag="G")
                    # 8-slot chunks: 1024 idxs = SWDGE ring capacity
                    for c in range(0, w, 8):
                        cw = min(8, w - c)
                        cni = 128 * cw
                        nc.gpsimd.dma_gather(
                            out_ap=G[:, c:c + cw, :],
                            in_ap=TBLS[li][:],
                            idxs_ap=sb_idx[:, ioff + 8 * c:ioff + 8 * (c + cw)],
                            num_idxs=cni, num_idxs_reg=cni, elem_size=TW,
                            queue_num=qctr % 4,
                        )
                        qctr += 1
                    asrc_own = cur[:, t, 37:43]
                    adst_own = cur[:, t, 43:49]
                    pay_own = cur[:, t, 0:37]

                    if li == 2:
                        # start the payload bf16 conversion on Scalar early
                        # so it overlaps the DVE logit work below
                        gb = big.tile([128, w, 37], BF16, tag="gb")
                        nc.scalar.activation(gb[:], G[:, :, 0:37],
                                             mybir.ActivationFunctionType.Copy)
                        pob = wrk.tile([128, 37], BF16, tag="pob")
                        nc.scalar.activation(pob[:], pay_own[:],
                                             mybir.ActivationFunctionType.Copy)

                    # logits: gathered slots + self slot, then leaky (Scalar)
                    lgall = wrk.tile([128, w + 1, H], F32, tag="lg")
                    nc.vector.tensor_tensor(
                        out=lgall[:, 0:w, :], in0=G[:, :, 37:43],
                        in1=adst_own[:, None, :].broadcast_to([128, w, H]),
                        op=mybir.AluOpType.add)
                    nc.vector.tensor_tensor(
                        out=lgall[:, w, :], in0=asrc_own, in1=adst_own,
                        op=mybir.AluOpType.add)
                    lgl = wrk.tile([128, w + 1, H], F32, tag="lgl")
                    nc.scalar.activation(lgl[:], lgall[:],
                                         mybir.ActivationFunctionType.Lrelu,
                                         alpha=0.2)

                    if li < 2:
                        # combined [ex(6) | msg(36)] buffer -> one tree pass
                        # yields den and agg together
                        em = wrk.tile([128, w + 1, H + HC], F32, tag="em")
                        ex = em[:, :, 0:H]
                        nc.scalar.activation(ex[:], lgl[:],
                                             mybir.ActivationFunctionType.Exp,
                                             bias=bm20[:])
                        nc.vector.tensor_tensor(
                            out=em[:, 0:w, H:H + HC]
                                .rearrange("p s (h c) -> p s h c", h=H),
                            in0=ex[:, 0:w, :][:, :, :, None]
                                .broadcast_to([128, w, H, H]),
                            in1=G[:, :, 0:HC].rearrange("p s (h c) -> p s h c", h=H),
                            op=mybir.AluOpType.mult)
                        nc.vector.tensor_tensor(
                            out=em[:, w, H:H + HC]
                                .rearrange("p (h c) -> p h c", h=H),
                            in0=ex[:, w, :][:, :, None].broadcast_to([128, H, H]),
                            in1=pay_own[:, 0:HC]
                                .rearrange("p (h c) -> p h c", h=H),
                            op=mybir.AluOpType.mult)
                        _tree_add(nc, em[:], w + 1)
                        den = em[:, 0, 0:H]
                        agg = em[:, 0, H:H + HC]
                        rd = wrk.tile([128, H], F32, tag="rd")
                        nc.vector.reciprocal(rd[:], den[:])
                        hp = wrk.tile([128, HC], F32, tag="hp")
                        nc.vector.tensor_tensor(
                            out=hp[:].rearrange("p (h c) -> p h c", h=H),
                            in0=agg[:].rearrange("p (h c) -> p h c", h=H),
                            in1=rd[:][:, :, None].broadcast_to([128, H, H]),
                            op=mybir.AluOpType.mult)
                        nc.vector.tensor_tensor(out=hp[:], in0=hp[:],
                                                in1=sb_b[li][:],
                                                op=mybir.AluOpType.add)
                        hr = wrk.tile([128, HC], F32, tag="hr")
                        nc.scalar.activation(hr[:], hp[:],
                                             mybir.ActivationFunctionType.Relu)
                        # inline next-layer table build for this tile
                        tp = ps.tile([HC, 128], F32, tag="tp")
                        nc.tensor.transpose(tp[:], hr[:], eye[:])
                        hs = wrk.tile([HC, 128], BF16, tag="hs")
                        nc.scalar.activation(hs[:], tp[:],
                                             mybir.ActivationFunctionType.Copy)
                        pt = ps.tile([128, 48], F32, tag="tb")
                        nc.tensor.matmul(pt[:], hs[:], sb_wc[li + 1][:],
                                         start=True, stop=True)
                        store_own(nxt, t, pt)
                        emit_tile(nxt, t)
                    else:
                        # bf16 message pipeline: payload and exp weights in
                        # bf16 (2x DVE), trees per chunk in bf16, partials
                        # accumulated in f32; logits stay f32 throughout
                        ex = wrk.tile([128, w + 1, H], BF16, tag="ex3")
                        nc.scalar.activation(ex[:], lgl[:],
                                             mybir.ActivationFunctionType.Exp,
                                             bias=bm20[:])
                        # acc3 [128, 6, 37]: init from self slot, then add
                        # chunked slot-trees; col 36 accumulates den
                        acc3 = wrk.tile([128, H, 37], F32, tag="acc3")
                        nc.vector.tensor_tensor(
                            out=acc3[:],
                            in0=ex[:, w, :][:, :, None].broadcast_to([128, H, 37]),
                            in1=pob[:][:, None, :].broadcast_to([128, H, 37]),
                            op=mybir.AluOpType.mult)
                        for c0 in range(0, w, CW3):
                            cw3 = min(CW3, w - c0)
                            m3 = big.tile([128, CW3, H, 37], BF16, tag="m3")
                            nc.vector.tensor_tensor(
                                out=m3[:, 0:cw3, :, :],
                                in0=ex[:, c0:c0 + cw3, :][:, :, :, None]
                                    .broadcast_to([128, cw3, H, 37]),
                                in1=gb[:, c0:c0 + cw3, :][:, :, None, :]
                                    .broadcast_to([128, cw3, H, 37]),
                                op=mybir.AluOpType.mult)
                            _tree_add(nc, m3[:, 0:cw3, :, :]
                                      .rearrange("p s h c -> p s (h c)"), cw3)
                            nc.vector.tensor_tensor(
                                out=acc3[:].rearrange("p h c -> p (h c)"),
                                in0=acc3[:].rearrange("p h c -> p (h c)"),
                                in1=m3[:, 0, :, :].rearrange("p h c -> p (h c)"),
                                op=mybir.AluOpType.add)
                        rd = wrk.tile([128, H], F32, tag="rd3")
                        nc.vector.reciprocal(rd[:], acc3[:, :, 36])
                        nc.vector.tensor_scalar_mul(rd[:], rd[:], 1.0 / 6.0)
                        aggn = wrk.tile([128, H, HC], F32, tag="aggn")
                        nc.vector.tensor_tensor(
                            out=aggn[:], in0=acc3[:, :, 0:HC],
                            in1=rd[:][:, :, None].broadcast_to([128, H, HC]),
                            op=mybir.AluOpType.mult)
                        zp = ps.tile([128, D_IN], F32, tag="z")
                        for h in range(H):
                            tp = ps.tile([HC, 128], F32, tag="tp")
                            nc.tensor.transpose(tp[:], aggn[:, h, :], eye[:])
                            ts = wrk.tile([HC, 128], BF16, tag="hs")
                            nc.scalar.activation(ts[:], tp[:],
                                                 mybir.ActivationFunctionType.Copy)
                            nc.tensor.matmul(zp[:], ts[:],
                                             sb_w3[:, h * 32:(h + 1) * 32],
                                             start=(h == 0), stop=(h == 5))
                        zs = wrk.tile([128, D_IN], F32, tag="zs")
                        nc.vector.tensor_tensor(out=zs[:], in0=zp[:],
                                                in1=sb_b[2][:],
                                                op=mybir.AluOpType.add)
                        nc.scalar.activation(e3_all[:, t, :], zs[:],
                                             mybir.ActivationFunctionType.Exp,
                                             bias=bm50[:])
                    ioff += 8 * w

                if li < 2:
                    tc.strict_bb_all_engine_barrier()
                    nc.gpsimd.collective_compute(
                        "AllGather", mybir.AluOpType.bypass,
                        replica_groups=[list(range(NC))],
                        ins=[BNC[:].opt()], outs=[TBLS[li + 1][:].opt()])
                    tc.strict_bb_all_engine_barrier()

            # ---- global softmax over nodes ----
            s0buf = wrk.tile([128, NT, D_IN], F32, tag="s0buf")
            nc.vector.tensor_tensor(
                out=s0buf[:], in0=e3_all[:],
                in1=rm[:][:, :, None].broadcast_to([128, NT, D_IN]),
                op=mybir.AluOpType.mult)
            _tree_add(nc, s0buf[:], NT)
            tp2 = ps.tile([D_IN, 128], F32, tag="tp")
            nc.tensor.transpose(tp2[:], s0buf[:, 0, :], eye[:])
            ts2 = wrk.tile([D_IN, 128], F32, tag="ts2")
            nc.scalar.activation(ts2[:], tp2[:],
                                 mybir.ActivationFunctionType.Copy)
            red = wrk.tile([D_IN, 1], F32, tag="red")
            nc.vector.tensor_reduce(out=red[:], in_=ts2[:],
                                    axis=mybir.AxisListType.X,
                                    op=mybir.AluOpType.add)
            nc.sync.dma_start(out=CCI[:], in_=red[:])
            tc.strict_bb_all_engine_barrier()
            nc.gpsimd.collective_compute(
                "AllReduce", mybir.AluOpType.add,
                replica_groups=[list(range(NC))],
                ins=[CCI[:].opt()], outs=[CCO[:].opt()])
            tc.strict_bb_all_engine_barrier()
            ssum = wrk.tile([D_IN, 1], F32, tag="ssum")
            nc.sync.dma_start(out=ssum[:], in_=CCO[:])
            rc32 = wrk.tile([D_IN, 1], F32, tag="rc32")
            nc.vector.reciprocal(rc32[:], ssum[:])
            rp1 = ps.tile([1, D_IN], F32, tag="tp")
            nc.tensor.transpose(rp1[:], rc32[:], eye[0:D_IN, 0:D_IN])
            rs1 = wrk.tile([1, D_IN], BF16, tag="rs1")
            nc.scalar.activation(rs1[:], rp1[:],
                                 mybir.ActivationFunctionType.Copy)
            rbp = ps.tile([128, D_IN], F32, tag="z")
            nc.tensor.matmul(rbp[:], ones1b[:], rs1[:], start=True, stop=True)
            rb = wrk.tile([128, D_IN], F32, tag="rb")
            nc.scalar.activation(rb[:], rbp[:],
                                 mybir.ActivationFunctionType.Copy)
            obf = wrk.tile([128, NT, D_IN], BF16, tag="obf")
            nc.vector.tensor_tensor(
                out=obf[:], in0=e3_all[:],
                in1=rb[:][:, None, :].broadcast_to([128, NT, D_IN]),
                op=mybir.AluOpType.mult)
            nc.sync.dma_start(
                out=t_out[:].rearrange("(t p) c -> p t c", p=128), in_=obf[:])
    return nc


def _make_runner(nc):
    """jit-compile the 8-core SPMD executable once; one dispatch per run,
    output buffer donated and recycled run-to-run."""
    import jax
    import numpy as _np
    from jax.sharding import Mesh, PartitionSpec, NamedSharding
    from concourse.bass2jax import (_bass_exec_p, partition_id_tensor,
                                    install_neuronx_cc_hook)

    install_neuronx_cc_hook()
    partition_name = nc.partition_id_tensor.name if nc.partition_id_tensor else None
    in_names, out_names, out_avals = [], [], []
    for alloc in nc.m.functions[0].allocations:
        if not isinstance(alloc, mybir.MemoryLocationSet):
            continue
        name = alloc.memorylocations[0].name
        if alloc.kind == "ExternalInput":
            if name != partition_name:
                in_names.append(name)
        elif alloc.kind == "ExternalOutput":
            out_names.append(name)
            shape = tuple(alloc.tensor_shape)
            dtype = mybir.dt.np(alloc.dtype)
            out_avals.append(jax.core.ShapedArray(shape, dtype))
    n_params = len(in_names)
    n_outs = len(out_avals)
    in_names_all = list(in_names) + out_names
    if partition_name is not None:
        in_names_all.append(partition_name)

    dbg_zero = None
    if nc.dbg_addr is not None:
        dbg_zero = _np.zeros((1, 2), _np.uint32)

    def _body(*args):
        operands = list(args)
        if partition_name is not None:
            operands.append(partition_id_tensor())
        outs = _bass_exec_p.bind(
            *operands,
            out_avals=tuple(out_avals),
            in_names=tuple(in_names_all),
            out_names=tuple(out_names),
            lowering_input_output_aliases=(),
            sim_require_finite=True,
            sim_require_nnan=True,
            nc=nc,
        )
        return tuple(outs)

    devices = jax.devices()[:NC]
    mesh = Mesh(_np.asarray(devices), ("core",))
    sharding = NamedSharding(mesh, PartitionSpec("core"))
    from jax.experimental.shard_map import shard_map
    sharded = jax.jit(
        shard_map(_body, mesh=mesh,
                  in_specs=(PartitionSpec("core"),) * (n_params + n_outs),
                  out_specs=(PartitionSpec("core"),) * n_outs,
                  check_rep=False),
        donate_argnums=tuple(range(n_params, n_params + n_outs)),
        keep_unused=True)

    def prepare(in_maps):
        maps = in_maps
        if dbg_zero is not None:
            maps = [{**m, nc.dbg_addr.name: dbg_zero} for m in maps]
        return [np.concatenate([np.asarray(maps[c][name])
                                for c in range(NC)], axis=0)
                for name in in_names]

    def make_outbuf():
        import ml_dtypes
        import jax as _jax
        return _jax.device_put(
            np.zeros((NC * NPP, D_IN), ml_dtypes.bfloat16), sharding)

    import jax as _jax

    def run(concat_in, out_buf):
        dev_in = [a if isinstance(a, _jax.Array) else _jax.device_put(a, sharding)
                  for a in concat_in]
        out_arrs = sharded(*dev_in, out_buf)
        o = out_arrs[out_names.index("out")]
        shards = sorted(o.addressable_shards,
                        key=lambda s: s.index[0].start or 0)
        for s in shards:
            s.data.copy_to_host_async()
        host = [np.asarray(s.data) for s in shards]
        return host, o

    return prepare, run, make_outbuf


_CACHE = {}
LAST_EXEC_NS = None     # HW (neuron-profile) exec time when available
LAST_E2E_NS = None      # host-measured steady-state end-to-end time
LAST_HW_NS = None


def _ntff_hw_time_ns(run_fn, args):
    """Profile one run via the axon NTFF hook; return max-core NEFF
    execution time in ns, or None if profiling is unavailable."""
    import ctypes, contextlib, glob, os, tempfile, subprocess, json, re
    so = "/opt/axon/libaxon_pjrt.so"
    if not os.path.exists(so):
        return None
    try:
        lib = ctypes.CDLL(so)
        if not hasattr(lib, "axon_start_nrt_profile"):
            return None
        lib.axon_start_nrt_profile.argtypes = [
            ctypes.POINTER(ctypes.c_int64), ctypes.c_size_t]
        lib.axon_start_nrt_profile.restype = ctypes.c_int64
        lib.axon_stop_nrt_profile.argtypes = [ctypes.c_char_p]
        lib.axon_stop_nrt_profile.restype = ctypes.c_int64
        outdir = tempfile.mkdtemp(prefix="ntff_")
        ids = (ctypes.c_int64 * NC)(*range(NC))
        rc = lib.axon_start_nrt_profile(ids, NC)
        if rc != 0:
            return None
        try:
            run_fn(*args)
        finally:
            n = lib.axon_stop_nrt_profile(outdir.encode())
        if n <= 0:
            return None
        ntffs = sorted(glob.glob(os.path.join(outdir, "*_body*.ntff")))
        neffs = glob.glob(os.path.join(outdir, "*_body*.neff"))
        if not ntffs or not neffs:
            return None
        times = []
        procs = []
        for f in ntffs:
            procs.append(subprocess.Popen(
                ["neuron-profile", "view", "-n", neffs[0], "-s", f,
                 "--output-format", "summary-json"],
                stdout=subprocess.PIPE, stderr=subprocess.DEVNULL))
        for p in procs:
            out, _ = p.communicate(timeout=300)
            m = re.search(rb'"total_time":([0-9.e-]+)', out)
            if m:
                times.append(float(m.group(1)))
        if not times or len(times) < NC:
            return None
        return int(max(times) * 1e9)
    except Exception:
        return None


def kernel(x, edge_index, W1, a_src1, a_dst1, b1, W2, a_src2, a_dst2, b2,
           W3, a_src3, a_dst3, b3):
    import time as _time
    global LAST_EXEC_NS, LAST_E2E_NS, LAST_HW_NS

    _tp0 = _time.time()
    x = np.asarray(x, np.float32)
    edge_index = np.asarray(edge_index)
    args = [np.asarray(a, np.float32) for a in
            (W1, a_src1, a_dst1, W2, a_src2, a_dst2, W3, a_src3, a_dst3)]
    bias = [np.asarray(a, np.float32) for a in (b1, b2, b3)]
    Wt, idx_w, blobbf, wblob, unperm = _prep(x, edge_index, *args, *bias)
    _tp1 = _time.time()
    print(f"[kernel] prep: {_tp1 - _tp0:.2f}s", flush=True)

    key = Wt.tobytes()
    if key in _CACHE:
        prepare, run, make_outbuf = _CACHE[key]
        _tp2 = _tp1
    else:
        nc = bacc.Bacc(None, num_devices=NC, num_swdge_queues=4)
        nc = _build(nc, Wt)
        nc.compile()
        prepare, run, make_outbuf = _make_runner(nc)
        _CACHE[key] = (prepare, run, make_outbuf)
        _tp2 = _time.time()
        print(f"[kernel] bass build+compile: {_tp2 - _tp1:.2f}s", flush=True)

    in_maps = [{"idxw": idx_w[k], "bbf": blobbf[k], "wbl": wblob}
               for k in range(NC)]
    concat_in = prepare(in_maps)

    # warmup (jit trace + NEFF compile/load, once per process)
    out_buf = make_outbuf()
    host, out_buf = run(concat_in, out_buf)
    host, out_buf = run(concat_in, out_buf)
    _tp3 = _time.time()
    print(f"[kernel] warmup: {_tp3 - _tp2:.2f}s", flush=True)

    # timed steady-state end-to-end runs (upload + exec + download)
    best_ns = None
    for _ in range(3):
        _t0 = _time.time()
        host, out_buf = run(concat_in, out_buf)
        ns = int((_time.time() - _t0) * 1e9)
        if best_ns is None or ns < best_ns:
            best_ns = ns
    LAST_E2E_NS = best_ns
    print(f"[kernel] e2e steady-state: {best_ns / 1e6:.1f} ms", flush=True)

    # hardware execution time via NTFF profiling (max across the 8 cores)
    hw_ns = _ntff_hw_time_ns(lambda: run(concat_in, out_buf), ())
    LAST_HW_NS = hw_ns
    if hw_ns is not None:
        print(f"[kernel] HW (NTFF) exec: {hw_ns / 1e3:.1f} us", flush=True)
        LAST_EXEC_NS = hw_ns
    else:
        print("[kernel] NTFF profiling unavailable; reporting e2e", flush=True)
        LAST_EXEC_NS = best_ns

    out = np.stack([np.asarray(s, np.float32) for s in host])  # [NC, NPP, 32]
    stacked = out[:, :NPC].reshape(NC * NPC, D_IN)
    full = np.zeros((N, D_IN), np.float32)
    full[unperm] = stacked
    return full


# revision 21
# speedup vs baseline: 1.2341x; 1.2341x over previous
"""GAT 3-layer (DiffusionOrderingNetwork) Trainium2 kernel, 8-core SPMD.

Strategy: nodes partitioned 8x2500 by dst; per-core ELL (degree-sorted,
per-tile width) edge layout; per-edge gathers via dma_gather 256B tokens
from a shared HBM node table [20480, 64] whose row layout is
[payload(36) | 1.0 | al_src(6) | al_dst(6) | pad]; each core builds only
its own 2560-row block and an AllGather assembles the full table per
layer. Self-loops are NOT materialized as edge slots: each dst's
self-contribution is computed locally from the core's own table rows.
Segment softmax uses a constant-shift exp (exact after normalization);
padding slots point at a NEG pad row whose al_src is -1e4 in the HBM
copy only, so their exp is exactly 0. The constant 1.0 table column
makes the layer-3 slot tree-reduction produce the softmax denominator
for free. Per-tile aggregation is an in-place contiguous halving tree
on DVE (no strided tensor_reduce). The next layer's table block is
built inline per tile (PE transpose + matmul on idle engines).

Host path: one jitted SPMD dispatch per run (donated output buffer is
recycled run-to-run; no separate zero-buffer dispatch), inputs are three
packed buffers per core (i16 idx stream, bf16 x+W1, bf16 weights), the
bf16 output shards are fetched concurrently. Reported HW exec time is
the NTFF (neuron-profile) max-core device execution time; the
steady-state host end-to-end time is also measured and reported.
"""

import sys

sys.path.insert(0, "/opt/trn_rl_repo")

import numpy as np
import concourse.bass as bass
import concourse.bacc as bacc
import concourse.mybir as mybir
import concourse.tile as tile
from concourse import library_config

N = 20000
NC = 8
NPC = 2500          # nodes per core
NT = 20             # node tiles per core (128 rows each)
NPP = NT * 128      # 2560 padded nodes per core
NTBL = NC * NPP     # 20480 table rows
H = 6
D_IN = 32
HC = 36             # heads * hidden
TW = 64             # table row width (f32) = 256B gather token
RW = 49             # used row width: 36 pay | 1.0 | 6 al_src | 6 al_dst
F32 = mybir.dt.float32
I16 = mybir.dt.int16
BF16 = mybir.dt.bfloat16

CBF = NPP + 48      # bf16 blob: xT_own | Wc1
BBF_XT = 0
BBF_WC1 = NPP
CW = 48 + 48 + 192 + 104  # bf16 weights blob: Wc2 | Wc3 | W3 | bias row
W_WC2 = 0
W_WC3 = 48
W_W3 = 96
W_B = 288

CW3 = 32            # layer-3 slot chunk for the outer-product buffer

NEGP = 96           # NEG pad rows live at partitions 96:128 of last tile
PADT = NT - 1


def _blockdiag(a):
    Hh, C = a.shape
    out = np.zeros((Hh * C, Hh), np.float32)
    for h in range(Hh):
        out[h * C:(h + 1) * C, h] = a[h]
    return out


def _prep(x, edge_index, W1, a_src1, a_dst1, W2, a_src2, a_dst2, W3, a_src3,
          a_dst3, b1, b2, b3):
    src = np.asarray(edge_index[0], np.int64)
    dst = np.asarray(edge_index[1], np.int64)

    deg = np.bincount(dst, minlength=N)
    orders = []          # per core: local pos -> global node id
    global_pos = np.zeros(N, np.int64)
    for k in range(NC):
        d = deg[k * NPC:(k + 1) * NPC]
        order = np.argsort(-d, kind="stable") + k * NPC
        orders.append(order)
        global_pos[order] = k * NPP + np.arange(NPC)

    # shared tile width schedule (max across cores), exact widths
    Wt = np.zeros(NT, np.int64)
    for k in range(NC):
        ds_ = np.sort(deg[k * NPC:(k + 1) * NPC])[::-1]
        ds_ = np.concatenate([ds_, np.zeros(NPP - NPC, np.int64)])
        Wt = np.maximum(Wt, ds_.reshape(NT, 128).max(axis=1))
    Wt = np.maximum(Wt, 1)

    # CSR by dst
    sort_by_dst = np.argsort(dst, kind="stable")
    src_s = src[sort_by_dst]
    rowptr = np.zeros(N + 1, np.int64)
    np.cumsum(deg, out=rowptr[1:])

    # ELL index stream per core: [16, CIDX] i16, pads -> NEG row
    idx_w = []
    for k in range(NC):
        order = orders[k]
        neg = k * NPP + PADT * 128 + NEGP
        iw_parts = []
        for t in range(NT):
            w = int(Wt[t])
            ell = np.full((128, w), neg, np.int64)
            for p in range(128):
                li = t * 128 + p
                if li < NPC:
                    n = order[li]
                    e0, e1 = rowptr[n], rowptr[n + 1]
                    ell[p, :e1 - e0] = global_pos[src_s[e0:e1]]
            stream = ell.T.reshape(-1)            # slot-major: s*128+p
            iw_parts.append(stream.reshape(-1, 16).T)  # [16, 8w]
        idx_w.append(np.concatenate(iw_parts, axis=1).astype(np.int16))

    # x in permuted order, padded, transposed: per-core [32, NPP]
    import ml_dtypes
    Wc1 = np.concatenate([W1, W1 @ _blockdiag(a_src1), W1 @ _blockdiag(a_dst1)], 1)
    Wc2 = np.concatenate([W2, W2 @ _blockdiag(a_src2), W2 @ _blockdiag(a_dst2)], 1)
    I36 = np.eye(HC, dtype=np.float32)
    Wc3 = np.concatenate([I36, W3 @ _blockdiag(a_src3), W3 @ _blockdiag(a_dst3)], 1)

    blobbf = []
    for k in range(NC):
        xp = np.zeros((NPP, D_IN), np.float32)
        xp[:NPC] = x[orders[k]]
        b = np.zeros((D_IN, CBF), ml_dtypes.bfloat16)
        b[:, BBF_XT:BBF_XT + NPP] = np.ascontiguousarray(xp.T).astype(ml_dtypes.bfloat16)
        b[:, BBF_WC1:BBF_WC1 + 48] = Wc1.astype(ml_dtypes.bfloat16)
        blobbf.append(b)

    wblob = np.zeros((HC, CW), ml_dtypes.bfloat16)
    wblob[:, W_WC2:W_WC2 + 48] = Wc2.astype(ml_dtypes.bfloat16)
    wblob[:, W_WC3:W_WC3 + 48] = Wc3.astype(ml_dtypes.bfloat16)
    wblob[:, W_W3:W_W3 + 192] = W3.astype(ml_dtypes.bfloat16)
    wblob[0, W_B:W_B + HC] = b1.astype(ml_dtypes.bfloat16)
    wblob[0, W_B + HC:W_B + 2 * HC] = b2.astype(ml_dtypes.bfloat16)
    wblob[0, W_B + 2 * HC:W_B + 2 * HC + D_IN] = b3.astype(ml_dtypes.bfloat16)

    unperm = np.concatenate(orders)  # row i of stacked core outputs -> node id
    return Wt, idx_w, blobbf, wblob, unperm


def _tree_add(nc, buf3, w):
    """In-place halving-tree sum over the middle (slot) axis of a
    [128, w, S] view; result lands in slot 0. Contiguous DVE adds."""
    while w > 1:
        k = w // 2
        nc.vector.tensor_tensor(out=buf3[:, 0:k, :], in0=buf3[:, 0:k, :],
                                in1=buf3[:, w - k:w, :],
                                op=mybir.AluOpType.add)
        w -= k


def _build(nc, Wt):
    SWt = int(Wt.sum())
    CIDX = 8 * SWt

    t_idx = nc.dram_tensor("idxw", [16, CIDX], I16, kind="ExternalInput")
    t_bf = nc.dram_tensor("bbf", [D_IN, CBF], BF16, kind="ExternalInput")
    t_w = nc.dram_tensor("wbl", [HC, CW], BF16, kind="ExternalInput")
    t_out = nc.dram_tensor("out", [NPP, D_IN], BF16, kind="ExternalOutput")

    with tile.TileContext(nc) as tc:
        with (
            tc.tile_pool(name="dram", bufs=1, space="DRAM") as dram,
            tc.tile_pool(name="cst", bufs=1) as cst,
            tc.tile_pool(name="gat", bufs=4) as gat,
            tc.tile_pool(name="wrk", bufs=3) as wrk,
            tc.tile_pool(name="big", bufs=2) as big,
            tc.tile_pool(name="acc", bufs=1) as acc,
            tc.tile_pool(name="ps", bufs=2, space="PSUM") as ps,
        ):
            nc.gpsimd.load_library(library_config.mlp)
            TBLS = [dram.tile([NTBL, TW], F32, addr_space="Shared",
                              name=f"tbls{i}", tag=f"tbls{i}") for i in range(3)]
            BNC = dram.tile([NPP, TW], F32)
            CCI = dram.tile([32, 1], F32)
            CCO = dram.tile([32, 1], F32)

            # idx stream: load once, replicate to 128 partitions on-chip
            sb_idx = cst.tile([128, CIDX], I16)
            nc.sync.dma_start(out=sb_idx[0:16, :], in_=t_idx[:])
            nc.sync.dma_start(out=sb_idx[16:32, :], in_=sb_idx[0:16, :])
            nc.sync.dma_start(out=sb_idx[32:64, :], in_=sb_idx[0:32, :])
            nc.sync.dma_start(out=sb_idx[64:128, :], in_=sb_idx[0:64, :])

            sb_bf = cst.tile([D_IN, CBF], BF16)
            nc.sync.dma_start(out=sb_bf[:], in_=t_bf[:])
            sb_w = cst.tile([HC, CW], BF16)
            nc.sync.dma_start(out=sb_w[:], in_=t_w[:])

            sb_xTo = sb_bf[:, BBF_XT:BBF_XT + NPP]
            sb_wc = [sb_bf[:, BBF_WC1:BBF_WC1 + 48],
                     sb_w[:, W_WC2:W_WC2 + 48],
                     sb_w[:, W_WC3:W_WC3 + 48]]
            sb_w3 = sb_w[:, W_W3:W_W3 + 192]

            # on-device constants: identity, row mask, biases
            eye = cst.tile([128, 128], F32)
            nc.gpsimd.memset(eye[:], 0.0)
            nc.gpsimd.affine_select(out=eye[:], in_=eye[:],
                                    compare_op=mybir.AluOpType.not_equal,
                                    fill=1.0, base=0, pattern=[[-1, 128]],
                                    channel_multiplier=1)
            rm = cst.tile([128, NT], F32)
            nc.gpsimd.memset(rm[:], 1.0)
            # keep where (NPC-1) - p - 128*t >= 0, i.e. real rows
            nc.gpsimd.affine_select(out=rm[:], in_=rm[:],
                                    compare_op=mybir.AluOpType.is_ge,
                                    fill=0.0, base=NPC - 1,
                                    pattern=[[-128, NT]],
                                    channel_multiplier=-1)
            ones1b = cst.tile([1, 128], BF16)
            nc.vector.memset(ones1b[:], 1.0)
            bias_ps = ps.tile([128, 104], F32, tag="tb")
            nc.tensor.matmul(bias_ps[:], ones1b[:], sb_w[0:1, W_B:W_B + 104],
                             start=True, stop=True)
            sb_bias = cst.tile([128, 104], F32)
            nc.scalar.activation(sb_bias[:], bias_ps[:],
                                 mybir.ActivationFunctionType.Copy)
            sb_b = [sb_bias[:, 0:HC], sb_bias[:, HC:2 * HC],
                    sb_bias[:, 2 * HC:2 * HC + D_IN]]

            bm20 = cst.tile([128, 1], F32)
            nc.vector.memset(bm20[:], -20.0)
            bm50 = cst.tile([128, 1], F32)
            nc.vector.memset(bm50[:], -50.0)

            # persistent own-rows table blocks (ping-pong across layers);
            # col 36 is the constant 1.0 column (written once, never touched)
            TBO = [acc.tile([128, NT, RW], F32, tag="tbo0", name="tbo0"),
                   acc.tile([128, NT, RW], F32, tag="tbo1", name="tbo1")]
            nc.vector.memset(TBO[0][:, :, 36], 1.0)
            nc.vector.memset(TBO[1][:, :, 36], 1.0)
            e3_all = acc.tile([128, NT, D_IN], F32)

            def store_own(dstbuf, t, pt):
                # psum row [pay(36) | al_src(6) | al_dst(6)] -> RW layout
                nc.scalar.activation(dstbuf[:, t, 0:36], pt[:, 0:36],
                                     mybir.ActivationFunctionType.Copy)
                nc.scalar.activation(dstbuf[:, t, 37:49], pt[:, 36:48],
                                     mybir.ActivationFunctionType.Copy)

            def emit_tile(dstbuf, t):
                # own block row -> BNC (HBM); NEG pad rows get al_src=-1e4
                # in the HBM copy only (local copy stays finite)
                if t == PADT:
                    sb_tb = wrk.tile([128, RW], F32, tag="tbpatch")
                    nc.vector.tensor_copy(sb_tb[:], dstbuf[:, t, :])
                    nc.vector.memset(sb_tb[NEGP:128, 37:43], -10000.0)
                    nc.sync.dma_start(out=BNC[t * 128:(t + 1) * 128, 0:RW],
                                      in_=sb_tb[:])
                else:
                    nc.sync.dma_start(out=BNC[t * 128:(t + 1) * 128, 0:RW],
                                      in_=dstbuf[:, t, :])

            # layer-1 table: xT (bf16) @ Wc1
            for t in range(NT):
                pt = ps.tile([128, 48], F32, tag="tb")
                nc.tensor.matmul(pt[:], sb_xTo[:, t * 128:(t + 1) * 128],
                                 sb_wc[0][:], start=True, stop=True)
                store_own(TBO[0], t, pt)
                emit_tile(TBO[0], t)
            tc.strict_bb_all_engine_barrier()
            nc.gpsimd.collective_compute(
                "AllGather", mybir.AluOpType.bypass,
                replica_groups=[list(range(NC))],
                ins=[BNC[:].opt()], outs=[TBLS[0][:].opt()])
            tc.strict_bb_all_engine_barrier()

            qctr = 0
            for li in range(3):
                cur = TBO[li % 2]
                nxt = TBO[(li + 1) % 2]
                ioff = 0
                for t in range(NT):
                    w = int(Wt[t])
                    G = gat.tile([128, w, TW], F32, tag="G")
                    # 8-slot chunks: 1024 idxs = SWDGE ring capacity
                    for c in range(0, w, 8):
                        cw = min(8, w - c)
                        cni = 128 * cw
                        nc.gpsimd.dma_gather(
                            out_ap=G[:, c:c + cw, :],
                            in_ap=TBLS[li][:],
                            idxs_ap=sb_idx[:, ioff + 8 * c:ioff + 8 * (c + cw)],
                            num_idxs=cni, num_idxs_reg=cni, elem_size=TW,
                            queue_num=qctr % 4,
                        )
                        qctr += 1
                    asrc_own = cur[:, t, 37:43]
                    adst_own = cur[:, t, 43:49]
                    pay_own = cur[:, t, 0:37]

                    if li == 2:
                        # start the payload bf16 conversion on Scalar early
                        # so it overlaps the DVE logit work below
                        gb = big.tile([128, w, 37], BF16, tag="gb")
                        nc.scalar.activation(gb[:], G[:, :, 0:37],
                                             mybir.ActivationFunctionType.Copy)
                        pob = wrk.tile([128, 37], BF16, tag="pob")
                        nc.scalar.activation(pob[:], pay_own[:],
                                             mybir.ActivationFunctionType.Copy)

                    # logits: gathered slots + self slot, then leaky (Scalar)
                    lgall = wrk.tile([128, w + 1, H], F32, tag="lg")
                    nc.vector.tensor_tensor(
                        out=lgall[:, 0:w, :], in0=G[:, :, 37:43],
                        in1=adst_own[:, None, :].broadcast_to([128, w, H]),
                        op=mybir.AluOpType.add)
                    nc.vector.tensor_tensor(
                        out=lgall[:, w, :], in0=asrc_own, in1=adst_own,
                        op=mybir.AluOpType.add)
                    lgs = wrk.tile([128, w + 1, H], F32, tag="lgs")
                    nc.vector.tensor_scalar_mul(lgs[:], lgall[:], 0.2)
                    nc.vector.tensor_max(lgall[:], lgall[:], lgs[:])

                    if li < 2:
                        # combined [ex(6) | msg(36)] buffer -> one tree pass
                        # yields den and agg together
                        em = wrk.tile([128, w + 1, H + HC], F32, tag="em")
                        ex = em[:, :, 0:H]
                        nc.scalar.activation(ex[:], lgall[:],
                                             mybir.ActivationFunctionType.Exp,
                                             bias=bm20[:])
                        nc.vector.tensor_tensor(
                            out=em[:, 0:w, H:H + HC]
                                .rearrange("p s (h c) -> p s h c", h=H),
                            in0=ex[:, 0:w, :][:, :, :, None]
                                .broadcast_to([128, w, H, H]),
                            in1=G[:, :, 0:HC].rearrange("p s (h c) -> p s h c", h=H),
                            op=mybir.AluOpType.mult)
                        nc.vector.tensor_tensor(
                            out=em[:, w, H:H + HC]
                                .rearrange("p (h c) -> p h c", h=H),
                            in0=ex[:, w, :][:, :, None].broadcast_to([128, H, H]),
                            in1=pay_own[:, 0:HC]
                                .rearrange("p (h c) -> p h c", h=H),
                            op=mybir.AluOpType.mult)
                        _tree_add(nc, em[:], w + 1)
                        den = em[:, 0, 0:H]
                        agg = em[:, 0, H:H + HC]
                        rd = wrk.tile([128, H], F32, tag="rd")
                        nc.vector.reciprocal(rd[:], den[:])
                        hp = wrk.tile([128, HC], F32, tag="hp")
                        nc.vector.tensor_tensor(
                            out=hp[:].rearrange("p (h c) -> p h c", h=H),
                            in0=agg[:].rearrange("p (h c) -> p h c", h=H),
                            in1=rd[:][:, :, None].broadcast_to([128, H, H]),
                            op=mybir.AluOpType.mult)
                        nc.vector.tensor_tensor(out=hp[:], in0=hp[:],
                                                in1=sb_b[li][:],
                                                op=mybir.AluOpType.add)
                        hr = wrk.tile([128, HC], F32, tag="hr")
                        nc.scalar.activation(hr[:], hp[:],
                                             mybir.ActivationFunctionType.Relu)
                        # inline next-layer table build for this tile
                        tp = ps.tile([HC, 128], F32, tag="tp")
                        nc.tensor.transpose(tp[:], hr[:], eye[:])
                        hs = wrk.tile([HC, 128], BF16, tag="hs")
                        nc.scalar.activation(hs[:], tp[:],
                                             mybir.ActivationFunctionType.Copy)
                        pt = ps.tile([128, 48], F32, tag="tb")
                        nc.tensor.matmul(pt[:], hs[:], sb_wc[li + 1][:],
                                         start=True, stop=True)
                        store_own(nxt, t, pt)
                        emit_tile(nxt, t)
                    else:
                        # bf16 message pipeline: payload and exp weights in
                        # bf16 (2x DVE), trees per chunk in bf16, partials
                        # accumulated in f32; logits stay f32 throughout
                        ex = wrk.tile([128, w + 1, H], BF16, tag="ex3")
                        nc.scalar.activation(ex[:], lgall[:],
                                             mybir.ActivationFunctionType.Exp,
                                             bias=bm20[:])
                        # acc3 [128, 6, 37]: init from self slot, then add
                        # chunked slot-trees; col 36 accumulates den
                        acc3 = wrk.tile([128, H, 37], F32, tag="acc3")
                        nc.vector.tensor_tensor(
                            out=acc3[:],
                            in0=ex[:, w, :][:, :, None].broadcast_to([128, H, 37]),
                            in1=pob[:][:, None, :].broadcast_to([128, H, 37]),
                            op=mybir.AluOpType.mult)
                        for c0 in range(0, w, CW3):
                            cw3 = min(CW3, w - c0)
                            m3 = big.tile([128, CW3, H, 37], BF16, tag="m3")
                            nc.vector.tensor_tensor(
                                out=m3[:, 0:cw3, :, :],
                                in0=ex[:, c0:c0 + cw3, :][:, :, :, None]
                                    .broadcast_to([128, cw3, H, 37]),
                                in1=gb[:, c0:c0 + cw3, :][:, :, None, :]
                                    .broadcast_to([128, cw3, H, 37]),
                                op=mybir.AluOpType.mult)
                            _tree_add(nc, m3[:, 0:cw3, :, :]
                                      .rearrange("p s h c -> p s (h c)"), cw3)
                            nc.vector.tensor_tensor(
                                out=acc3[:].rearrange("p h c -> p (h c)"),
                                in0=acc3[:].rearrange("p h c -> p (h c)"),
                                in1=m3[:, 0, :, :].rearrange("p h c -> p (h c)"),
                                op=mybir.AluOpType.add)
                        rd = wrk.tile([128, H], F32, tag="rd3")
                        nc.vector.reciprocal(rd[:], acc3[:, :, 36])
                        nc.vector.tensor_scalar_mul(rd[:], rd[:], 1.0 / 6.0)
                        aggn = wrk.tile([128, H, HC], F32, tag="aggn")
                        nc.vector.tensor_tensor(
                            out=aggn[:], in0=acc3[:, :, 0:HC],
                            in1=rd[:][:, :, None].broadcast_to([128, H, HC]),
                            op=mybir.AluOpType.mult)
                        zp = ps.tile([128, D_IN], F32, tag="z")
                        for h in range(H):
                            tp = ps.tile([HC, 128], F32, tag="tp")
                            nc.tensor.transpose(tp[:], aggn[:, h, :], eye[:])
                            ts = wrk.tile([HC, 128], BF16, tag="hs")
                            nc.scalar.activation(ts[:], tp[:],
                                                 mybir.ActivationFunctionType.Copy)
                            nc.tensor.matmul(zp[:], ts[:],
                                             sb_w3[:, h * 32:(h + 1) * 32],
                                             start=(h == 0), stop=(h == 5))
                        zs = wrk.tile([128, D_IN], F32, tag="zs")
                        nc.vector.tensor_tensor(out=zs[:], in0=zp[:],
                                                in1=sb_b[2][:],
                                                op=mybir.AluOpType.add)
                        nc.scalar.activation(e3_all[:, t, :], zs[:],
                                             mybir.ActivationFunctionType.Exp,
                                             bias=bm50[:])
                    ioff += 8 * w

                if li < 2:
                    tc.strict_bb_all_engine_barrier()
                    nc.gpsimd.collective_compute(
                        "AllGather", mybir.AluOpType.bypass,
                        replica_groups=[list(range(NC))],
                        ins=[BNC[:].opt()], outs=[TBLS[li + 1][:].opt()])
                    tc.strict_bb_all_engine_barrier()

            # ---- global softmax over nodes ----
            s0buf = wrk.tile([128, NT, D_IN], F32, tag="s0buf")
            nc.vector.tensor_tensor(
                out=s0buf[:], in0=e3_all[:],
                in1=rm[:][:, :, None].broadcast_to([128, NT, D_IN]),
                op=mybir.AluOpType.mult)
            _tree_add(nc, s0buf[:], NT)
            tp2 = ps.tile([D_IN, 128], F32, tag="tp")
            nc.tensor.transpose(tp2[:], s0buf[:, 0, :], eye[:])
            ts2 = wrk.tile([D_IN, 128], F32, tag="ts2")
            nc.scalar.activation(ts2[:], tp2[:],
                                 mybir.ActivationFunctionType.Copy)
            red = wrk.tile([D_IN, 1], F32, tag="red")
            nc.vector.tensor_reduce(out=red[:], in_=ts2[:],
                                    axis=mybir.AxisListType.X,
                                    op=mybir.AluOpType.add)
            nc.sync.dma_start(out=CCI[:], in_=red[:])
            tc.strict_bb_all_engine_barrier()
            nc.gpsimd.collective_compute(
                "AllReduce", mybir.AluOpType.add,
                replica_groups=[list(range(NC))],
                ins=[CCI[:].opt()], outs=[CCO[:].opt()])
            tc.strict_bb_all_engine_barrier()
            ssum = wrk.tile([D_IN, 1], F32, tag="ssum")
            nc.sync.dma_start(out=ssum[:], in_=CCO[:])
            rc32 = wrk.tile([D_IN, 1], F32, tag="rc32")
            nc.vector.reciprocal(rc32[:], ssum[:])
            rp1 = ps.tile([1, D_IN], F32, tag="tp")
            nc.tensor.transpose(rp1[:], rc32[:], eye[0:D_IN, 0:D_IN])
            rs1 = wrk.tile([1, D_IN], BF16, tag="rs1")
            nc.scalar.activation(rs1[:], rp1[:],
                                 mybir.ActivationFunctionType.Copy)
            rbp = ps.tile([128, D_IN], F32, tag="z")
            nc.tensor.matmul(rbp[:], ones1b[:], rs1[:], start=True, stop=True)
            rb = wrk.tile([128, D_IN], F32, tag="rb")
            nc.scalar.activation(rb[:], rbp[:],
                                 mybir.ActivationFunctionType.Copy)
            obf = wrk.tile([128, NT, D_IN], BF16, tag="obf")
            nc.vector.tensor_tensor(
                out=obf[:], in0=e3_all[:],
                in1=rb[:][:, None, :].broadcast_to([128, NT, D_IN]),
                op=mybir.AluOpType.mult)
            nc.sync.dma_start(
                out=t_out[:].rearrange("(t p) c -> p t c", p=128), in_=obf[:])
    return nc


def _make_runner(nc):
    """jit-compile the 8-core SPMD executable once; one dispatch per run,
    output buffer donated and recycled run-to-run."""
    import jax
    import numpy as _np
    from jax.sharding import Mesh, PartitionSpec, NamedSharding
    from concourse.bass2jax import (_bass_exec_p, partition_id_tensor,
                                    install_neuronx_cc_hook)

    install_neuronx_cc_hook()
    partition_name = nc.partition_id_tensor.name if nc.partition_id_tensor else None
    in_names, out_names, out_avals = [], [], []
    for alloc in nc.m.functions[0].allocations:
        if not isinstance(alloc, mybir.MemoryLocationSet):
            continue
        name = alloc.memorylocations[0].name
        if alloc.kind == "ExternalInput":
            if name != partition_name:
                in_names.append(name)
        elif alloc.kind == "ExternalOutput":
            out_names.append(name)
            shape = tuple(alloc.tensor_shape)
            dtype = mybir.dt.np(alloc.dtype)
            out_avals.append(jax.core.ShapedArray(shape, dtype))
    n_params = len(in_names)
    n_outs = len(out_avals)
    in_names_all = list(in_names) + out_names
    if partition_name is not None:
        in_names_all.append(partition_name)

    dbg_zero = None
    if nc.dbg_addr is not None:
        dbg_zero = _np.zeros((1, 2), _np.uint32)

    def _body(*args):
        operands = list(args)
        if partition_name is not None:
            operands.append(partition_id_tensor())
        outs = _bass_exec_p.bind(
            *operands,
            out_avals=tuple(out_avals),
            in_names=tuple(in_names_all),
            out_names=tuple(out_names),
            lowering_input_output_aliases=(),
            sim_require_finite=True,
            sim_require_nnan=True,
            nc=nc,
        )
        return tuple(outs)

    devices = jax.devices()[:NC]
    mesh = Mesh(_np.asarray(devices), ("core",))
    sharding = NamedSharding(mesh, PartitionSpec("core"))
    from jax.experimental.shard_map import shard_map
    sharded = jax.jit(
        shard_map(_body, mesh=mesh,
                  in_specs=(PartitionSpec("core"),) * (n_params + n_outs),
                  out_specs=(PartitionSpec("core"),) * n_outs,
                  check_rep=False),
        donate_argnums=tuple(range(n_params, n_params + n_outs)),
        keep_unused=True)

    def prepare(in_maps):
        maps = in_maps
        if dbg_zero is not None:
            maps = [{**m, nc.dbg_addr.name: dbg_zero} for m in maps]
        return [np.concatenate([np.asarray(maps[c][name])
                                for c in range(NC)], axis=0)
                for name in in_names]

    def make_outbuf():
        import ml_dtypes
        import jax as _jax
        return _jax.device_put(
            np.zeros((NC * NPP, D_IN), ml_dtypes.bfloat16), sharding)

    import jax as _jax

    def run(concat_in, out_buf):
        dev_in = [a if isinstance(a, _jax.Array) else _jax.device_put(a, sharding)
                  for a in concat_in]
        out_arrs = sharded(*dev_in, out_buf)
        o = out_arrs[out_names.index("out")]
        shards = sorted(o.addressable_shards,
                        key=lambda s: s.index[0].start or 0)
        for s in shards:
            s.data.copy_to_host_async()
        host = [np.asarray(s.data) for s in shards]
        return host, o

    return prepare, run, make_outbuf


_CACHE = {}
LAST_EXEC_NS = None     # HW (neuron-profile) exec time when available
LAST_E2E_NS = None      # host-measured steady-state end-to-end time
LAST_HW_NS = None


def _ntff_hw_time_ns(run_fn, args):
    """Profile one run via the axon NTFF hook; return max-core NEFF
    execution time in ns, or None if profiling is unavailable."""
    import ctypes, contextlib, glob, os, tempfile, subprocess, json, re
    so = "/opt/axon/libaxon_pjrt.so"
    if not os.path.exists(so):
        return None
    try:
        lib = ctypes.CDLL(so)
        if not hasattr(lib, "axon_start_nrt_profile"):
            return None
        lib.axon_start_nrt_profile.argtypes = [
            ctypes.POINTER(ctypes.c_int64), ctypes.c_size_t]
        lib.axon_start_nrt_profile.restype = ctypes.c_int64
        lib.axon_stop_nrt_profile.argtypes = [ctypes.c_char_p]
        lib.axon_stop_nrt_profile.restype = ctypes.c_int64
        outdir = tempfile.mkdtemp(prefix="ntff_")
        ids = (ctypes.c_int64 * NC)(*range(NC))
        rc = lib.axon_start_nrt_profile(ids, NC)
        if rc != 0:
            return None
        try:
            run_fn(*args)
        finally:
            n = lib.axon_stop_nrt_profile(outdir.encode())
        if n <= 0:
            return None
        ntffs = sorted(glob.glob(os.path.join(outdir, "*_body*.ntff")))
        neffs = glob.glob(os.path.join(outdir, "*_body*.neff"))
        if not ntffs or not neffs:
            return None
        times = []
        procs = []
        for f in ntffs:
            procs.append(subprocess.Popen(
                ["neuron-profile", "view", "-n", neffs[0], "-s", f,
                 "--output-format", "summary-json"],
                stdout=subprocess.PIPE, stderr=subprocess.DEVNULL))
        for p in procs:
            out, _ = p.communicate(timeout=300)
            m = re.search(rb'"total_time":([0-9.e-]+)', out)
            if m:
                times.append(float(m.group(1)))
        if not times or len(times) < NC:
            return None
        return int(max(times) * 1e9)
    except Exception:
        return None


def kernel(x, edge_index, W1, a_src1, a_dst1, b1, W2, a_src2, a_dst2, b2,
           W3, a_src3, a_dst3, b3):
    import time as _time
    global LAST_EXEC_NS, LAST_E2E_NS, LAST_HW_NS

    _tp0 = _time.time()
    x = np.asarray(x, np.float32)
    edge_index = np.asarray(edge_index)
    args = [np.asarray(a, np.float32) for a in
            (W1, a_src1, a_dst1, W2, a_src2, a_dst2, W3, a_src3, a_dst3)]
    bias = [np.asarray(a, np.float32) for a in (b1, b2, b3)]
    Wt, idx_w, blobbf, wblob, unperm = _prep(x, edge_index, *args, *bias)
    _tp1 = _time.time()
    print(f"[kernel] prep: {_tp1 - _tp0:.2f}s", flush=True)

    key = Wt.tobytes()
    if key in _CACHE:
        prepare, run, make_outbuf = _CACHE[key]
        _tp2 = _tp1
    else:
        nc = bacc.Bacc(None, num_devices=NC, num_swdge_queues=4)
        nc = _build(nc, Wt)
        nc.compile()
        prepare, run, make_outbuf = _make_runner(nc)
        _CACHE[key] = (prepare, run, make_outbuf)
        _tp2 = _time.time()
        print(f"[kernel] bass build+compile: {_tp2 - _tp1:.2f}s", flush=True)

    in_maps = [{"idxw": idx_w[k], "bbf": blobbf[k], "wbl": wblob}
               for k in range(NC)]
    concat_in = prepare(in_maps)

    # warmup (jit trace + NEFF compile/load, once per process)
    out_buf = make_outbuf()
    host, out_buf = run(concat_in, out_buf)
    host, out_buf = run(concat_in, out_buf)
    _tp3 = _time.time()
    print(f"[kernel] warmup: {_tp3 - _tp2:.2f}s", flush=True)

    # timed steady-state end-to-end runs (upload + exec + download)
    best_ns = None
    for _ in range(3):
        _t0 = _time.time()
        host, out_buf = run(concat_in, out_buf)
        ns = int((_time.time() - _t0) * 1e9)
        if best_ns is None or ns < best_ns:
            best_ns = ns
    LAST_E2E_NS = best_ns
    print(f"[kernel] e2e steady-state: {best_ns / 1e6:.1f} ms", flush=True)

    # hardware execution time via NTFF profiling (max across the 8 cores)
    hw_ns = _ntff_hw_time_ns(lambda: run(concat_in, out_buf), ())
    LAST_HW_NS = hw_ns
    if hw_ns is not None:
        print(f"[kernel] HW (NTFF) exec: {hw_ns / 1e3:.1f} us", flush=True)
        LAST_EXEC_NS = hw_ns
    else:
        print("[kernel] NTFF profiling unavailable; reporting e2e", flush=True)
        LAST_EXEC_NS = best_ns

    out = np.stack([np.asarray(s, np.float32) for s in host])  # [NC, NPP, 32]
    stacked = out[:, :NPC].reshape(NC * NPC, D_IN)
    full = np.zeros((N, D_IN), np.float32)
    full[unperm] = stacked
    return full
